# revision 1
# baseline (speedup 1.0000x reference)
"""Trainium2 Bass kernel for nn_BraidCrossing (B=8, T=2048, D=2048, NG=3).

Math notes
----------
reference computes:
    pair  = [x_t, x_{t+1}]                       (B, T-1, 2D)
    h     = gelu(pair @ W1.T + b1)
    logit = h @ W2.T + b2                        (B, T-1, 2*NG)
    scale = mean(softmax(logit, -1), -1)         == 1/(2*NG) EXACTLY (mean of a
                                                 softmax over the same axis)
    P     = x @ Wp.T + bp
    tmp_t = LN(x_t + P_{t-1} * scale)   t>=1 ;  tmp_0 = x_0
    out_t = LN(tmp_t + P_{t+1} * scale) t<=T-2; out_{T-1} = tmp_{T-1}

Because scale is a constant (1/(2*NG); setup has bp=0, gamma=1, beta=0), the
entire W1/W2/gelu branch is dead code.  The device kernel computes
Q = (x @ Wp.T) * scale with an f32r matmul, then the two chained layernorms.

Sharding: data-parallel over batch, one batch per NeuronCore (8 cores).
Per core: x[b] (natural), x[b].T (for the matmul lhsT), Wp.T (resident).
"""
import numpy as np

import concourse.bass as bass
from concourse import bacc
import concourse.mybir as mybir
import concourse.tile as tile
from concourse.bass_utils import run_bass_kernel_spmd

FP32 = mybir.dt.float32
F32R = mybir.dt.float32r
AF = mybir.ActivationFunctionType
ALU = mybir.AluOpType

B, T, D = 8, 2048, 2048
P = 128                # partitions
NT = T // P            # 16 t-tiles
NE = D // 512          # 4 psum-bank chunks along e
EPS = 1e-5
N_CORES = 8

_cache = {}


def _build(scale: float):
    nc = bacc.Bacc("TRN2", target_bir_lowering=False, debug=False)
    x_d = nc.declare_dram_parameter("x", [T, D], FP32, isOutput=False)
    # host-tiled transpose: xTt[i, p, k, tt] = x[i*128+tt, k*128+p], so the
    # per-t-tile lhsT load is one fully contiguous 1 MiB DMA (8 KiB/partition)
    xTt_d = nc.declare_dram_parameter("xTt", [NT, P, NT, P], F32R, isOutput=False)
    wT_d = nc.declare_dram_parameter("wT", [D, D], F32R, isOutput=False)
    out_d = nc.declare_dram_parameter("out", [T, D], FP32, isOutput=True)

    x_ap = x_d.ap()
    out_ap = out_d.ap()
    xTt_ap = xTt_d.ap()

    with tile.TileContext(nc) as tc:
        with tc.tile_pool(name="wp", bufs=1) as wp_pool, \
             tc.tile_pool(name="xt", bufs=3) as xt_pool, \
             tc.tile_pool(name="q", bufs=2) as q_pool, \
             tc.tile_pool(name="v", bufs=4) as v_pool, \
             tc.tile_pool(name="stat", bufs=4) as stat_pool, \
             tc.tile_pool(name="ps", bufs=8, space="PSUM") as ps_pool:

            eps_t = stat_pool.tile([P, 1], FP32, tag="eps", bufs=1)
            nc.vector.memset(eps_t, EPS)

            # resident Wp.T: 16 k-tiles of (128, 2048) f32r, split across two
            # HWDGE queues so the 16 MiB stream drains in parallel
            wp = []
            H = D // 2
            for k in range(NT):
                w = wp_pool.tile([P, D], F32R, tag=f"wp{k}", bufs=1)
                nc.sync.dma_start(out=w[:, 0:H],
                                  in_=wT_d.ap()[k * P:(k + 1) * P, 0:H])
                nc.scalar.dma_start(out=w[:, H:D],
                                    in_=wT_d.ap()[k * P:(k + 1) * P, H:D])
                wp.append(w)

            # prefetch the first two t-tiles' lhsT AFTER the weights on the
            # sync queue: the first matmul then starts only once the weight
            # stream has drained, so the PE burst is dense (HAM stays warm)
            # instead of trickling cold behind weight arrivals.
            xt_pre = {}
            for i in (NT - 1, NT - 2):
                xt_i = xt_pool.tile([P, NT, P], F32R, tag="xt")
                nc.sync.dma_start(out=xt_i, in_=xTt_ap[i])
                xt_pre[i] = xt_i


            def layer_norm_inplace(v, nparts, apply_chunks=1, apply_dve=False):
                """v[:nparts] = (v - mean(v)) * rsqrt(var(v) + eps), per row."""
                stats = stat_pool.tile([P, NE, 6], FP32, tag="stats")
                mv = stat_pool.tile([P, 2], FP32, tag="mv")
                for c in range(NE):
                    nc.vector.bn_stats(out=stats[:nparts, c, :],
                                       in_=v[:nparts, c * 512:(c + 1) * 512])
                nc.vector.bn_aggr(out=mv[:nparts], in_=stats[:nparts])
                rs = stat_pool.tile([P, 1], FP32, tag="rs")
                nc.scalar.activation(out=rs[:nparts], in_=mv[:nparts, 1:2],
                                     func=AF.Sqrt, bias=eps_t[:nparts], scale=1.0)
                nc.vector.reciprocal(out=rs[:nparts], in_=rs[:nparts])
                cw = D // apply_chunks
                if apply_dve:
                    # DVE apply: v = (v - mu) * rs  (used near the kernel tail
                    # where the scalar engine is the congested one)
                    for c in range(apply_chunks):
                        nc.vector.tensor_scalar(
                            out=v[:nparts, c * cw:(c + 1) * cw],
                            in0=v[:nparts, c * cw:(c + 1) * cw],
                            scalar1=mv[:nparts, 0:1], scalar2=rs[:nparts],
                            op0=ALU.subtract, op1=ALU.mult)
                    return
                # ACT apply: v = Identity(v*rs + (-mu*rs))
                nmr = stat_pool.tile([P, 1], FP32, tag="nmr")
                nc.vector.scalar_tensor_tensor(out=nmr[:nparts],
                                               in0=mv[:nparts, 0:1], scalar=-1.0,
                                               in1=rs[:nparts],
                                               op0=ALU.mult, op1=ALU.mult)
                for c in range(apply_chunks):
                    nc.scalar.activation(out=v[:nparts, c * cw:(c + 1) * cw],
                                         in_=v[:nparts, c * cw:(c + 1) * cw],
                                         func=AF.Identity, bias=nmr[:nparts],
                                         scale=rs[:nparts])

            def chunked_add(dst, src, nparts, chunks):
                cw = D // chunks
                for c in range(chunks):
                    nc.vector.tensor_add(out=dst[:nparts, c * cw:(c + 1) * cw],
                                         in0=dst[:nparts, c * cw:(c + 1) * cw],
                                         in1=src[:nparts, c * cw:(c + 1) * cw])

            # Tiles are processed in REVERSE order (15 .. 0): tile i's v2 tail
            # rows need Q[(i+1)*128 .. +1], i.e. q_{i+1}, which in reverse
            # order was produced the PREVIOUS iteration — so every tile
            # finishes inside its own iteration (no cross-iteration chain).
            q_next = None     # q tile of i+1 (previous iteration)

            for i in reversed(range(NT)):
                ns = P if i < NT - 1 else P - 1    # valid rows of v1/tmp

                xt_i = xt_pre.pop(i)

                # Q tile i: (128 t, 2048 e) = scale * x[t-rows] @ Wp.T
                # one 4-bank PSUM tile; one ACT copy per tile
                qp = ps_pool.tile([P, D], FP32, tag="qps", bufs=2)
                for n in range(NE):
                    for k in range(NT):
                        nc.tensor.matmul(qp[:, n * 512:(n + 1) * 512],
                                         xt_i[:, k, :],
                                         wp[k][:, n * 512:(n + 1) * 512],
                                         start=(k == 0), stop=(k == NT - 1))

                # prefetch the NEXT tile's lhsT immediately after the matmuls
                # so its DMA sits ahead of this tile's LN-chain semaphores in
                # the queue (emission order = queue order)
                if i - 1 >= 0 and (i - 1) not in xt_pre:
                    xt_n = xt_pool.tile([P, NT, P], F32R, tag="xt")
                    nc.sync.dma_start(out=xt_n, in_=xTt_ap[i - 1])
                    xt_pre[i - 1] = xt_n

                q_i = q_pool.tile([P, D], FP32, tag="q")
                nc.scalar.activation(out=q_i[:], in_=qp[:], func=AF.Copy,
                                     scale=scale)

                # the last-processed tile's chain is the kernel tail: emit its
                # adds/applies column-chunked so the LN stages soft-pipeline
                nchunk = 4 if i <= 1 else 1
                tail_tile = i <= 1

                # v1 = x[i*128+1 : ...] + Q[u],  tmp = LN(v1) in place
                v1 = v_pool.tile([P, D], FP32, tag="v")
                nc.sync.dma_start(out=v1[:ns, :],
                                  in_=x_ap[i * P + 1: i * P + 1 + ns, :])
                if tail_tile:
                    # read Q straight from PSUM so v1 does not wait on the
                    # ACT q-copy (which only v2 needs)
                    for c in range(NE):
                        sl = slice(c * 512, (c + 1) * 512)
                        nc.vector.scalar_tensor_tensor(
                            out=v1[:ns, sl], in0=qp[:ns, sl], scalar=scale,
                            in1=v1[:ns, sl], op0=ALU.mult, op1=ALU.add)
                else:
                    chunked_add(v1, q_i, ns, nchunk)
                layer_norm_inplace(v1, ns, apply_chunks=(2 if tail_tile else 1),
                                   apply_dve=tail_tile)
                tmp_i = v1  # row r = tmp[i*128 + r + 1]

                # v2 = tmp + Q[u+2]; rows 0..125 from q_i[2:128],
                # rows 126,127 from q_{i+1}[0:2] (loaded last iteration)
                no2 = P if i < NT - 1 else P - 2
                v2 = v_pool.tile([P, D], FP32, tag="v")
                nc.gpsimd.dma_start(out=v2[0:126, :], in_=q_i[2:P, :])
                if i < NT - 1:
                    nc.gpsimd.dma_start(out=v2[126:128, :], in_=q_next[0:2, :])
                chunked_add(v2, tmp_i, no2, nchunk)
                layer_norm_inplace(v2, no2, apply_chunks=(2 if tail_tile else 1),
                                   apply_dve=tail_tile)
                nc.gpsimd.dma_start(out=out_ap[i * P + 1: i * P + 1 + no2, :],
                                    in_=v2[:no2, :])

                if i == NT - 1:
                    # out[T-1] = tmp[T-1] = tmp_15 row 126
                    nc.gpsimd.dma_start(out=out_ap[T - 1:T, :],
                                        in_=tmp_i[126:127, :])
                if i == 0:
                    # out[0] = LN(x[0] + Q[1]): 1-row boundary
                    qrow1 = v_pool.tile([1, D], FP32, tag="v")
                    nc.sync.dma_start(out=qrow1, in_=q_i[1:2, :])
                    v0 = v_pool.tile([1, D], FP32, tag="v")
                    nc.sync.dma_start(out=v0[0:1, :], in_=x_ap[0:1, :])
                    nc.vector.tensor_add(out=v0[0:1, :], in0=v0[0:1, :],
                                         in1=qrow1[0:1, :])
                    layer_norm_inplace(v0, 1, apply_dve=True)
                    nc.sync.dma_start(out=out_ap[0:1, :], in_=v0[0:1, :])

                q_next = q_i

    nc.compile()
    return nc


def _get_program(scale: float):
    key = round(float(scale), 9)
    if key not in _cache:
        _cache[key] = _build(float(scale))
    return _cache[key]


def _identity_ln_params(bp, gamma, beta):
    return (not np.any(bp)) and (not np.any(beta)) and np.all(gamma == 1.0)


def _reference_numpy(x, W1, b1, W2, b2, Wp, bp, gamma, beta):
    """Exact numpy port of the jax reference (emergency fallback only)."""
    import math

    def ln(v):
        mu = v.mean(-1, keepdims=True)
        var = ((v - mu) ** 2).mean(-1, keepdims=True)
        return (v - mu) / np.sqrt(var + EPS) * gamma + beta

    erf = np.vectorize(math.erf)
    x64 = x.astype(np.float32)
    pair = np.concatenate([x64[:, :-1], x64[:, 1:]], axis=-1)
    h0 = pair @ W1.T + b1
    h = 0.5 * h0 * (1.0 + erf(h0 / np.sqrt(2.0)))
    logits = h @ W2.T + b2
    e = np.exp(logits - logits.max(-1, keepdims=True))
    sm = e / e.sum(-1, keepdims=True)
    scale = sm.mean(-1, keepdims=True)
    Pm = x64 @ Wp.T + bp
    m = Pm[:, 1:] * scale
    mp = Pm[:, :-1] * scale
    tmp = np.concatenate([x64[:, :1], ln(x64[:, 1:] + mp)], axis=1)
    out = np.concatenate([ln(tmp[:, :-1] + m), tmp[:, -1:]], axis=1)
    return out.astype(np.float32)


def run_device(x, wT, scale, trace=False):
    """x: (B,T,D) fp32, wT: (D,D) fp32 (= Wp.T contiguous)."""
    nc = _get_program(scale)
    in_maps = []
    for c in range(N_CORES):
        xb = np.ascontiguousarray(x[c])
        # xTt[i, p, k, tt] = x[i*128+tt, k*128+p]
        xTb = np.ascontiguousarray(
            x[c].reshape(NT, P, NT, P).transpose(0, 3, 2, 1))
        in_maps.append({"x": xb, "xTt": xTb, "wT": wT})
    res = run_bass_kernel_spmd(nc, in_maps, list(range(N_CORES)), trace=trace)
    out = np.stack([res.results[c]["out"] for c in range(N_CORES)], axis=0)
    return out, res


def kernel(x, W1, b1, W2, b2, Wp, bp, gamma, beta):
    x = np.asarray(x, dtype=np.float32)
    Wp = np.asarray(Wp, dtype=np.float32)
    bp = np.asarray(bp); gamma = np.asarray(gamma); beta = np.asarray(beta)
    b2 = np.asarray(b2)
    if x.shape != (B, T, D) or not _identity_ln_params(bp, gamma, beta):
        return _reference_numpy(np.asarray(x), np.asarray(W1), np.asarray(b1),
                                np.asarray(W2), b2, Wp, bp, gamma, beta)
    scale = 1.0 / float(b2.shape[0])
    wT = np.ascontiguousarray(Wp.T)
    out, _ = run_device(x, wT, scale, trace=False)
    return out

